# revision 7
# baseline (speedup 1.0000x reference)
"""Trainium2 Bass kernel for MLA-attention + MoE transformer block.

Distribution (8 NeuronCores, SPMD, two launches):
  Launch A: attention sharded by (batch, head): core c handles batch c//4,
            heads 3*(c%4)..3*(c%4)+2. All-transposed layouts ([feature, token])
            so every matmul contracts on the partition dim with no transposes.
  Host:     assembles x_attn = x + a@wo from the 8 partial outputs, does
            rmsnorm2 + gate logits + exact top-2 routing + token dispatch
            (the "all-to-all") in numpy (tiny: [1024,8]).
  Launch B: expert-parallel MoE: core e runs expert e on its gathered tokens
            (max 284 of avg 256; capacity 320 with chunked fallback).
Matmuls run in float32r (full PE rate; ~1.6e-4 rel err). Routing decisions are
driven by logits whose error vs the fp32 reference is ~1e-6, far below the
min top2/top3 prob gap (1.4e-5 for these fixed inputs), so routing matches
the reference exactly; everything else is continuous.
"""

import time

import numpy as np
import concourse.bass as bass
import concourse.mybir as mybir
from concourse import bacc
from concourse.bass_utils import run_bass_kernel_spmd
from concourse.tile import TileContext

_times = {}


def _timed_run(label, nc, in_maps, core_ids):
    t0 = time.perf_counter()
    res = run_bass_kernel_spmd(nc, in_maps, core_ids)
    _times.setdefault(label, []).append(time.perf_counter() - t0)
    return res

AF = mybir.ActivationFunctionType
F32 = mybir.dt.float32
F32R = mybir.dt.float32r

B, S, D, H, Dh, DL, E, HID = 2, 512, 768, 12, 64, 64, 8, 4 * 768
T = B * S
P = 128
DK = D // P          # 6
HM = HID // P        # 24
HPC = 3              # heads per core
CAP = 320            # expert token capacity per launch-B run
NCORE = 8
EPS = 1e-6
ROPE_BASE = 10000.0

_cache = {}


# ---------------------------------------------------------------- launch A
def _build_a():
    nc = bacc.Bacc(None)
    xT = nc.declare_dram_parameter("xT", [P, DK, S], F32, isOutput=False)
    n1w = nc.declare_dram_parameter("n1w", [P, DK, 1], F32, isOutput=False)
    wq3 = nc.declare_dram_parameter("wq3", [P, HPC, DK, Dh], F32R, isOutput=False)
    wdkv = nc.declare_dram_parameter("wdkv", [P, DK, DL], F32R, isOutput=False)
    wuk3 = nc.declare_dram_parameter("wuk3", [DL, HPC, Dh], F32R, isOutput=False)
    wuv3 = nc.declare_dram_parameter("wuv3", [DL, HPC, Dh], F32R, isOutput=False)
    wo3 = nc.declare_dram_parameter("wo3", [Dh, HPC, DK, P], F32R, isOutput=False)
    cs2 = nc.declare_dram_parameter("cs2", [Dh, S], F32, isOutput=False)
    sn2 = nc.declare_dram_parameter("sn2", [Dh, S], F32, isOutput=False)
    mask = nc.declare_dram_parameter("mask", [P, 4, S], F32, isOutput=False)
    awoT = nc.declare_dram_parameter("awoT", [DK, P, S], F32, isOutput=True)

    with TileContext(nc) as tc:
        with tc.tile_pool(name="const", bufs=1) as cp, \
             tc.tile_pool(name="big", bufs=1) as bp, \
             tc.tile_pool(name="head", bufs=2) as hp, \
             tc.tile_pool(name="psum", bufs=1, space="PSUM") as ps:
            # PSUM budget (8 banks): roto(1) qk(2) sc(2) colsum(1) aps(1) csb(1)
            ones_f = cp.tile([P, 1], F32)
            nc.vector.memset(ones_f[:], 1.0)
            ones_r = cp.tile([P, 1], F32R)
            nc.vector.tensor_copy(ones_r[:], ones_f[:])
            ones_row = cp.tile([1, P], F32)
            nc.vector.memset(ones_row[:], 1.0)

            x_t = bp.tile([P, DK, S], F32)
            nc.sync.dma_start(x_t[:], xT[:])
            n1_t = cp.tile([P, DK, 1], F32)
            nc.sync.dma_start(n1_t[:], n1w[:])
            cs_t = cp.tile([Dh, S], F32)
            nc.sync.dma_start(cs_t[:], cs2[:])
            sn_t = cp.tile([Dh, S], F32)
            nc.sync.dma_start(sn_t[:], sn2[:])
            mask_t = bp.tile([P, 4, S], F32)
            nc.sync.dma_start(mask_t[:], mask[:])
            wq_t = bp.tile([P, HPC, DK, Dh], F32R)
            nc.sync.dma_start(wq_t[:], wq3[:])
            wdkv_t = bp.tile([P, DK, DL], F32R)
            nc.sync.dma_start(wdkv_t[:], wdkv[:])
            wuk_t = cp.tile([DL, HPC, Dh], F32R)
            nc.sync.dma_start(wuk_t[:], wuk3[:])
            wuv_t = cp.tile([DL, HPC, Dh], F32R)
            nc.sync.dma_start(wuv_t[:], wuv3[:])
            wo_t = bp.tile([Dh, HPC, DK, P], F32R)
            nc.sync.dma_start(wo_t[:], wo3[:])

            # ---- rmsnorm1 (transposed): h = x * rsqrt(mean(x^2)+eps) * w ----
            xsq = bp.tile([P, DK, S], F32R)
            for k in range(DK):
                nc.vector.tensor_mul(xsq[:, k], x_t[:, k], x_t[:, k])
            ss_ps = ps.tile([1, S], F32, tag="roto")
            for k in range(DK):
                nc.tensor.matmul(ss_ps[:], ones_r[:], xsq[:, k],
                                 start=(k == 0), stop=(k == DK - 1))
            ms = cp.tile([1, S], F32)
            nc.scalar.activation(ms[:], ss_ps[:], AF.Copy, bias=EPS, scale=1.0 / D)
            rec = cp.tile([1, S], F32)
            nc.vector.reciprocal(rec[:], ms[:])
            invr = cp.tile([1, S], F32)
            nc.scalar.activation(invr[:], rec[:], AF.Sqrt)
            inv_ps = ps.tile([P, S], F32, tag="roto")
            nc.tensor.matmul(inv_ps[:], ones_row[:], invr[:], start=True, stop=True)
            inv_b = bp.tile([P, S], F32)
            nc.scalar.copy(inv_b[:], inv_ps[:])
            h_tr = bp.tile([P, DK, S], F32R)
            hf = bp.tile([P, DK, S], F32)
            for k in range(DK):
                nc.vector.tensor_mul(hf[:, k], x_t[:, k], inv_b[:])
                nc.vector.tensor_scalar_mul(h_tr[:, k], hf[:, k], n1_t[:, k])

            # ---- c = h @ w_dkv  (cT [64, S]) ----
            c_ps = ps.tile([DL, S], F32, tag="roto")
            for k in range(DK):
                nc.tensor.matmul(c_ps[:], wdkv_t[:, k], h_tr[:, k],
                                 start=(k == 0), stop=(k == DK - 1))
            c_sb = cp.tile([DL, S], F32R)
            nc.vector.tensor_copy(c_sb[:], c_ps[:])

            a_norm = [None] * HPC
            for j in range(HPC):
                # qT/kT [64, S]
                q_ps = ps.tile([Dh, S], F32, tag="qk")
                for k in range(DK):
                    nc.tensor.matmul(q_ps[:], wq_t[:, j, k], h_tr[:, k],
                                     start=(k == 0), stop=(k == DK - 1))
                k_ps = ps.tile([Dh, S], F32, tag="qk")
                nc.tensor.matmul(k_ps[:], wuk_t[:, j], c_sb[:], start=True, stop=True)
                # v [S, 64] (token-major), per 128-token chunk
                v_sb = hp.tile([P, 4, Dh], F32R, tag="v")
                for kc in range(4):
                    v_ps = ps.tile([P, Dh], F32, tag="roto")
                    nc.tensor.matmul(v_ps[:], c_sb[:, kc * P:(kc + 1) * P],
                                     wuv_t[:, j], start=True, stop=True)
                    nc.any.tensor_copy(v_sb[:, kc], v_ps[:])
                # rope: rot = t*cs2 + shuf(t)*sn2 ; shuf swaps halves
                qf = hp.tile([Dh, S], F32, tag="qf")
                kf = hp.tile([Dh, S], F32, tag="kf")
                nc.any.tensor_copy(qf[:], q_ps[:])
                nc.any.tensor_copy(kf[:], k_ps[:])
                qs = hp.tile([Dh, S], F32, tag="qs")
                ks = hp.tile([Dh, S], F32, tag="ks")
                half = Dh // 2
                nc.sync.dma_start(qs[:half], qf[half:])
                nc.sync.dma_start(qs[half:], qf[:half])
                nc.sync.dma_start(ks[:half], kf[half:])
                nc.sync.dma_start(ks[half:], kf[:half])
                qrot = hp.tile([Dh, S], F32R, tag="qrot")
                krot = hp.tile([Dh, S], F32R, tag="krot")
                t1 = hp.tile([Dh, S], F32, tag="t1")
                t2 = hp.tile([Dh, S], F32, tag="t2")
                nc.vector.tensor_mul(t1[:], qf[:], cs_t[:])
                nc.any.tensor_mul(t2[:], qs[:], sn_t[:])
                nc.vector.tensor_add(qrot[:], t1[:], t2[:])
                t3 = hp.tile([Dh, S], F32, tag="t1")
                t4 = hp.tile([Dh, S], F32, tag="t2")
                nc.vector.tensor_mul(t3[:], kf[:], cs_t[:])
                nc.any.tensor_mul(t4[:], ks[:], sn_t[:])
                nc.vector.tensor_add(krot[:], t3[:], t4[:])

                # scoresT chunks + exp + mask; colsum + av accumulate
                attn = hp.tile([P, 4, S], F32R, tag="attn")
                cs_ps2 = ps.tile([1, S], F32, tag="colsum")
                a_ps = ps.tile([Dh, S], F32, tag="aps")
                for kc in range(4):
                    sc_ps = ps.tile([P, S], F32, tag="sc")
                    nc.tensor.matmul(sc_ps[:], krot[:, kc * P:(kc + 1) * P], qrot[:],
                                     start=True, stop=True)
                    ex = hp.tile([P, S], F32, tag="ex")
                    nc.scalar.activation(ex[:], sc_ps[:], AF.Exp,
                                         scale=float(1.0 / np.sqrt(Dh)))
                    nc.vector.tensor_mul(attn[:, kc], ex[:], mask_t[:, kc])
                    nc.tensor.matmul(cs_ps2[:], ones_r[:], attn[:, kc],
                                     start=(kc == 0), stop=(kc == 3))
                    nc.tensor.matmul(a_ps[:], v_sb[:, kc], attn[:, kc],
                                     start=(kc == 0), stop=(kc == 3))
                csr = hp.tile([1, S], F32, tag="csr")
                nc.vector.reciprocal(csr[:], cs_ps2[:])
                csb_ps = ps.tile([Dh, S], F32, tag="csb")
                nc.tensor.matmul(csb_ps[:], ones_row[:, :Dh], csr[:],
                                 start=True, stop=True)
                csb_sb = hp.tile([Dh, S], F32, tag="csb_sb", name="csb_sb")
                nc.any.tensor_copy(csb_sb[:], csb_ps[:])
                a_norm[j] = hp.tile([Dh, S], F32R, tag=f"anorm{j}", name=f"anorm{j}")
                nc.vector.tensor_mul(a_norm[j][:], a_ps[:], csb_sb[:])

            # ---- awo partial: awoT[m] = sum_j wo_j[:, m].T @ a_norm[j] ----
            for m in range(DK):
                awo_ps = ps.tile([P, S], F32, tag="qk")
                for j in range(HPC):
                    nc.tensor.matmul(awo_ps[:], wo_t[:, j, m], a_norm[j][:],
                                     start=(j == 0), stop=(j == HPC - 1))
                awo_sb = hp.tile([P, S], F32, tag="awo_sb", name="awo_sb")
                nc.any.tensor_copy(awo_sb[:], awo_ps[:])
                nc.sync.dma_start(awoT[m], awo_sb[:])
    nc.compile()
    return nc


# ---------------------------------------------------------------- launch B
def _build_b():
    nc = bacc.Bacc(None)
    xeT = nc.declare_dram_parameter("xeT", [P, DK, CAP], F32R, isOutput=False)
    w1m = nc.declare_dram_parameter("w1m", [HM, P, DK, P], F32R, isOutput=False)
    w3m = nc.declare_dram_parameter("w3m", [HM, P, DK, P], F32R, isOutput=False)
    w2m = nc.declare_dram_parameter("w2m", [P, HM, DK, P], F32R, isOutput=False)
    yT = nc.declare_dram_parameter("yT", [DK, P, CAP], F32, isOutput=True)

    with TileContext(nc) as tc:
        with tc.tile_pool(name="xe", bufs=1) as xp, \
             tc.tile_pool(name="w13", bufs=3) as wp, \
             tc.tile_pool(name="w2", bufs=1) as w2p, \
             tc.tile_pool(name="hid", bufs=1) as hidp, \
             tc.tile_pool(name="sl", bufs=3) as slp, \
             tc.tile_pool(name="psum", bufs=2, space="PSUM") as ps:
            # PSUM budget (8 banks): h1(2) h3(2) y(2)
            xe_t = xp.tile([P, DK, CAP], F32R)
            nc.sync.dma_start(xe_t[:], xeT[:])
            w2_t = w2p.tile([P, HM, DK, P], F32R)
            nc.sync.dma_start(w2_t[:], w2m[:])

            # pass 1: hid[m] = silu(w1_m.T @ xe) * (w3_m.T @ xe), all m
            hid = [None] * HM
            for m in range(HM):
                w1_t = wp.tile([P, DK, P], F32R, tag="w1")
                nc.sync.dma_start(w1_t[:], w1m[m])
                w3_t = wp.tile([P, DK, P], F32R, tag="w3")
                nc.sync.dma_start(w3_t[:], w3m[m])
                h1 = ps.tile([P, CAP], F32, tag="h1")
                for k in range(DK):
                    nc.tensor.matmul(h1[:], w1_t[:, k], xe_t[:, k],
                                     start=(k == 0), stop=(k == DK - 1))
                h3 = ps.tile([P, CAP], F32, tag="h3")
                for k in range(DK):
                    nc.tensor.matmul(h3[:], w3_t[:, k], xe_t[:, k],
                                     start=(k == 0), stop=(k == DK - 1))
                sl = slp.tile([P, CAP], F32, tag="sl")
                nc.scalar.activation(sl[:], h1[:], AF.Silu)
                hid[m] = hidp.tile([P, CAP], F32R, tag=f"hid{m}", name=f"hid{m}")
                nc.vector.tensor_mul(hid[m][:], sl[:], h3[:])

            # pass 2: yT[m2] = sum_m w2[:, m, m2].T @ hid[m]
            for m2 in range(DK):
                y_ps = ps.tile([P, CAP], F32, tag="y", name="y_ps")
                for m in range(HM):
                    nc.tensor.matmul(y_ps[:], w2_t[:, m, m2], hid[m][:],
                                     start=(m == 0), stop=(m == HM - 1))
                y_sb = slp.tile([P, CAP], F32, tag="y_sb", name="y_sb")
                nc.any.tensor_copy(y_sb[:], y_ps[:])
                nc.sync.dma_start(yT[m2], y_sb[:])
    nc.compile()
    return nc


# ---------------------------------------------------------------- host glue
def _rope_tables(start_pos):
    inv_freq = (1.0 / (ROPE_BASE ** (np.arange(0, Dh, 2, dtype=np.float32) / Dh)))
    inv_freq = inv_freq.astype(np.float32)
    pos = np.arange(S, dtype=np.float32) + np.float32(start_pos)
    ang = (pos[:, None] * inv_freq[None, :]).astype(np.float32)   # [S, 32]
    cos = np.cos(ang).astype(np.float32).T                        # [32, S]
    sin = np.sin(ang).astype(np.float32).T
    cs2 = np.concatenate([cos, cos], axis=0)                      # [64, S]
    sn2 = np.concatenate([-sin, sin], axis=0)                     # [64, S]
    return np.ascontiguousarray(cs2), np.ascontiguousarray(sn2)


def _to_pkn(a2d):
    # [R*128, N] -> [128, R, N]  (feature-major to partition-chunk layout)
    R = a2d.shape[0] // P
    return np.ascontiguousarray(a2d.reshape(R, P, -1).transpose(1, 0, 2))


def _get_programs():
    if "a" not in _cache:
        _cache["a"] = _build_a()
    if "b" not in _cache:
        _cache["b"] = _build_b()
    return _cache["a"], _cache["b"]


def kernel(x, norm1_w, norm2_w, wq, w_dkv, w_uk, w_uv, wo, gate_w, w1, w2, w3,
           start_pos):
    x = np.asarray(x, dtype=np.float32)
    nc_a, nc_b = _get_programs()

    # ---------------- launch A inputs ----------------
    cs2, sn2 = _rope_tables(int(start_pos))
    maskT = (np.arange(S)[:, None] <= np.arange(S)[None, :]).astype(np.float32)
    mask_l = np.ascontiguousarray(maskT.reshape(4, P, S).transpose(1, 0, 2))
    n1_l = _to_pkn(np.asarray(norm1_w, np.float32).reshape(D, 1))
    wdkv_l = _to_pkn(np.asarray(w_dkv, np.float32))
    in_maps_a = []
    for c in range(NCORE):
        b = c // 4
        heads = [3 * (c % 4) + j for j in range(HPC)]
        xT_l = _to_pkn(np.ascontiguousarray(x[b].T))          # [128, 6, 512]
        wq_l = np.stack([_to_pkn(np.asarray(wq, np.float32)[:, h * Dh:(h + 1) * Dh])
                         for h in heads], axis=1)              # [128, 3, 6, 64]
        wuk_l = np.stack([np.asarray(w_uk, np.float32)[:, h * Dh:(h + 1) * Dh]
                          for h in heads], axis=1)             # [64, 3, 64]
        wuv_l = np.stack([np.asarray(w_uv, np.float32)[:, h * Dh:(h + 1) * Dh]
                          for h in heads], axis=1)
        wo_l = np.stack([np.asarray(wo, np.float32)[h * Dh:(h + 1) * Dh]
                         .reshape(Dh, DK, P) for h in heads], axis=1)  # [64,3,6,128]
        in_maps_a.append({
            "xT": xT_l, "n1w": n1_l, "wq3": np.ascontiguousarray(wq_l),
            "wdkv": wdkv_l, "wuk3": np.ascontiguousarray(wuk_l),
            "wuv3": np.ascontiguousarray(wuv_l), "wo3": np.ascontiguousarray(wo_l),
            "cs2": cs2, "sn2": sn2, "mask": mask_l,
        })
    res_a = _timed_run("A", nc_a, in_maps_a, list(range(NCORE)))

    # ---------------- host: assemble x_attn, routing, dispatch ----------------
    awo = np.zeros((B, S, D), np.float32)
    for c in range(NCORE):
        part = res_a.results[c]["awoT"]                       # [6, 128, 512]
        awo[c // 4] += part.reshape(D, S).T
    x_attn = x + awo                                          # [B, S, D]
    xf2 = x_attn.reshape(T, D)
    msq = np.mean(xf2 * xf2, axis=-1, keepdims=True, dtype=np.float32)
    h2 = (xf2 * (1.0 / np.sqrt(msq + np.float32(EPS))).astype(np.float32)
          * np.asarray(norm2_w, np.float32)[None, :]).astype(np.float32)
    logits = (h2 @ np.asarray(gate_w, np.float32)).astype(np.float32)
    lmax = logits.max(axis=-1, keepdims=True)
    pe_ = np.exp(logits - lmax, dtype=np.float32)
    probs = (pe_ / pe_.sum(axis=-1, keepdims=True, dtype=np.float32)).astype(np.float32)
    order = np.argsort(-probs, axis=-1, kind="stable")
    topi = order[:, :2]                                       # [T, 2]
    topv = np.take_along_axis(probs, topi, axis=-1)
    topn = (topv / topv.sum(axis=-1, keepdims=True)).astype(np.float32)

    # aux loss (fp32, matching reference formula)
    counts = np.zeros(E, np.float32)
    for kk in range(2):
        counts += np.bincount(topi[:, kk], minlength=E)
    f = (counts / np.float32(T * 2)).astype(np.float32) * np.float32(2)
    Pm = probs.mean(axis=0, dtype=np.float32).astype(np.float32)
    aux = np.float32(E) * np.float32(np.sum(f * Pm, dtype=np.float32))

    # dispatch
    h2T = np.ascontiguousarray(h2.T)                          # [D, T]
    tok_lists = [np.where((topi[:, 0] == e) | (topi[:, 1] == e))[0] for e in range(E)]
    w1_np = np.asarray(w1, np.float32)
    w2_np = np.asarray(w2, np.float32)
    w3_np = np.asarray(w3, np.float32)
    y_total = np.zeros((T, D), np.float32)
    max_n = max(len(t) for t in tok_lists)
    n_chunks = max(1, (max_n + CAP - 1) // CAP)
    for chunk in range(n_chunks):
        in_maps_b = []
        chunk_toks = []
        for e in range(E):
            toks = tok_lists[e][chunk * CAP:(chunk + 1) * CAP]
            chunk_toks.append(toks)
            xe = np.zeros((D, CAP), np.float32)
            xe[:, :len(toks)] = h2T[:, toks]
            in_maps_b.append({
                "xeT": _to_pkn(xe),
                "w1m": np.ascontiguousarray(
                    w1_np[e].reshape(DK, P, HM, P).transpose(2, 1, 0, 3)),
                "w3m": np.ascontiguousarray(
                    w3_np[e].reshape(DK, P, HM, P).transpose(2, 1, 0, 3)),
                "w2m": np.ascontiguousarray(
                    w2_np[e].reshape(HM, P, DK, P).transpose(1, 0, 2, 3)),
            })
        res_b = _timed_run("B", nc_b, in_maps_b, list(range(NCORE)))
        # combine in expert order (matches reference accumulation order)
        for e in range(E):
            toks = chunk_toks[e]
            if len(toks) == 0:
                continue
            ye = res_b.results[e]["yT"].reshape(D, CAP).T[:len(toks)]  # [n, D]
            w_e = np.where(topi[toks, 0] == e, topn[toks, 0], topn[toks, 1])
            y_total[toks] += w_e[:, None].astype(np.float32) * ye

    out = (x_attn + y_total.reshape(B, S, D)).astype(np.float32)
    return out, np.float32(aux)
